# revision 1
# baseline (speedup 1.0000x reference)
"""NetVLAD Trainium2 Bass kernel.

Full inputs in, full output out. Data-parallel over batch N=64 across 8
NeuronCores (8 samples per core); conv weight and centroids replicated.

Per-sample algorithm (mathematically equal to the reference, never
materializing the channel-normalized x):
  X = x[n]  [D=128, P=4800]  (D on SBUF partitions, contiguous in HBM)
  For each 128-wide chunk of P (p on partitions after a PE transpose):
    ss[p]    = sum_d X[d,p]^2
    inv_s    = ss^-0.5                   (DVE pow — keeps the ACT table
                                          set fixed: only Copy/Square/Exp)
    logitsT  = X_c^T @ Wt                (PE)
    e        = exp(logitsT * inv_s)      (softmax max-subtraction skipped:
                                          |logits*inv_s| <= ~1.2)
    sb       = e * (inv_s / Z),  Z = sum_k e
    acc     += [sb | e]^T @ [X_c^T | 1/Z]   (PE, PSUM accumulate)
  agg      = acc[0:64, 0:128];  sum_sa = acc[64:128, 128]
  vlad     = agg - sum_sa * centroids, then intra + global L2 norm.

Pipelining: per-chunk scalar work is batched into whole-sample ops
(one Square, one reduce, one Exp, ...); the accumulate matmuls of
sample n-2 are emitted between pass A of sample n so the PE never
waits on the scalar chain. The [sb|e] and [XT|1/Z] operands are bf16
(FWL fast weight load; f32 PSUM accumulation).
"""

import sys

if "/opt/trn_rl_repo" not in sys.path:
    sys.path.insert(0, "/opt/trn_rl_repo")

import numpy as np
from contextlib import ExitStack

N, D, HW, K = 64, 128, 4800, 64
NCORES = 8
NS = N // NCORES  # samples per core

CHUNKS = [(i * 128, min(128, HW - i * 128)) for i in range((HW + 127) // 128)]
NCH = len(CHUNKS)  # 38: 37 full + one 64-wide

_CACHE = {}


def _patch_act_tables():
    """Steer bacc's ACT table-set placement to the one set that covers
    every function we use (ln/exp/square/copy) so the kernel pays a single
    ACT_TABLE_LOAD instead of thrashing between per-anchor sets."""
    if _CACHE.get("act_patched"):
        return
    from concourse import bacc, mybir

    orig = bacc.get_activation_tables
    AF = mybir.ActivationFunctionType
    combo = "natural_log_exp_and_others"

    def patched(arch):
        t = {k: set(v) for k, v in orig(arch).items()}
        if combo in t:
            for name in t:
                if name != combo:
                    t[name] = t[name] - {AF.Ln, AF.Exp}
        return t

    bacc.get_activation_tables = patched
    _CACHE["act_patched"] = True


def _build_nc():
    import concourse.tile as tile
    from concourse import bacc, mybir

    _patch_act_tables()

    nc = bacc.Bacc(
        "TRN2",
        target_bir_lowering=False,
        debug=False,
        enable_asserts=False,
        num_devices=NCORES,
    )
    x_ap = nc.dram_tensor("x", [NS, D, HW], mybir.dt.float32, kind="ExternalInput").ap()
    wt_ap = nc.dram_tensor("wt", [D, K], mybir.dt.float32, kind="ExternalInput").ap()
    cent_ap = nc.dram_tensor(
        "cent", [K, D], mybir.dt.float32, kind="ExternalInput"
    ).ap()
    out_ap = nc.dram_tensor(
        "out", [NS, K, D], mybir.dt.float32, kind="ExternalOutput"
    ).ap()

    with tile.TileContext(nc) as tc:
        with ExitStack() as ctx:
            _body(ctx, tc, out_ap, x_ap, wt_ap, cent_ap)
    nc.compile()
    return nc


def _body(ctx, tc, out_ap, x_ap, wt_ap, cent_ap):
    import concourse.bass as bass
    from concourse import masks, mybir

    nc = tc.nc
    f32 = mybir.dt.float32
    bf16 = mybir.dt.bfloat16
    AF = mybir.ActivationFunctionType
    ALU = mybir.AluOpType
    X_AX = mybir.AxisListType.X

    singles = ctx.enter_context(tc.tile_pool(name="singles", bufs=1))
    xpool = ctx.enter_context(tc.tile_pool(name="xpool", bufs=2))
    xtrpool = ctx.enter_context(tc.tile_pool(name="xtrpool", bufs=4))
    ebpool = ctx.enter_context(tc.tile_pool(name="ebpool", bufs=2))
    sbtpool = ctx.enter_context(tc.tile_pool(name="sbtpool", bufs=4))
    lpool = ctx.enter_context(tc.tile_pool(name="lpool", bufs=2))
    scrpool = ctx.enter_context(tc.tile_pool(name="scrpool", bufs=2))
    smalls = ctx.enter_context(tc.tile_pool(name="smalls", bufs=3))
    tails = ctx.enter_context(tc.tile_pool(name="tails", bufs=1))
    pp_xt = ctx.enter_context(tc.tile_pool(name="pp_xt", bufs=6, space="PSUM"))
    pp_acc = ctx.enter_context(tc.tile_pool(name="pp_acc", bufs=1, space="PSUM"))
    pp_tiny = ctx.enter_context(tc.tile_pool(name="pp_tiny", bufs=1, space="PSUM"))

    def bcast(ap, n):
        # append a step-0 free dim: [..., n] broadcast view
        return bass.AP(tensor=ap.tensor, offset=ap.offset, ap=list(ap.ap) + [[0, n]])

    def mid_bcast(ap, n):
        # [p, f] -> [p, n, f] with step-0 middle dim
        return bass.AP(
            tensor=ap.tensor,
            offset=ap.offset,
            ap=[ap.ap[0], [0, n]] + list(ap.ap[1:]),
        )

    # constants
    ident = singles.tile([128, 128], f32)
    masks.make_identity(nc, ident[:])
    # fused rhs for pass A: [identity | Wt] — one matmul yields [X_c^T | logits]
    identwt = singles.tile([128, 192], f32)
    masks.make_identity(nc, identwt[:, 0:128])
    nc.sync.dma_start(out=identwt[:, 128:192], in_=wt_ap[:])
    cent_s = singles.tile([K, D], f32)
    nc.sync.dma_start(out=cent_s[:], in_=cent_ap[:])
    ones_col = singles.tile([K, 1], f32)
    nc.vector.memset(ones_col[:], 1.0)
    ones_row = singles.tile([1, K], f32)
    nc.vector.memset(ones_row[:], 1.0)

    GRP = 2  # fused-matmul chunks per PSUM bank (finer -> more PE runway)
    groups = []
    c0 = 0
    while c0 < NCH:
        groups.append(list(range(c0, min(c0 + GRP, NCH))))
        c0 += GRP

    state = {}  # per-sample live tiles

    def emit_load_and_passA(n, cpass=None):
        xs = xpool.tile([D, HW], f32, tag="xs")
        nc.sync.dma_start(out=xs[:, 0 : HW // 2], in_=x_ap[n, :, 0 : HW // 2])
        nc.sync.dma_start(out=xs[:, HW // 2 :], in_=x_ap[n, :, HW // 2 :])

        # [XT | s] per chunk (bf16): cols 0:128 = X_c^T, col 128 = ||x_p||
        xtr = xtrpool.tile([128, NCH, 129], bf16, tag="xtr")
        # softmax numerators e (bf16, contiguous for the 2x reduce)
        et = ebpool.tile([128, NCH, K], bf16, tag="et")
        # sb = e * inv_s/Z — the acc matmul's stationary operand
        sbt = sbtpool.tile([128, NCH, K], bf16, tag="sbt")
        # raw logits stash (bf16)
        lgs = lpool.tile([128, NCH, K], bf16, tag="lgs")
        # XT^2 scratch (bf16 — ss reduce gets the 2x DVE mode)
        x2t = scrpool.tile([128, NCH * 128], bf16, tag="x2t")
        # scaled-logits scratch (bf16)
        slgt = scrpool.tile([128, NCH * K], bf16, tag="slgt")

        for gi, grp in enumerate(groups):
            gn = len(grp)
            # one fused matmul per chunk: out cols 0:128 = X_c^T, 128:192 =
            # logits. 256-col stride keeps each 192-col output in one bank.
            xt_p = pp_xt.tile([128, GRP, 256], f32, tag="xt")
            for j, c in enumerate(grp):
                p0, w = CHUNKS[c]
                x_c = xs[:, p0 : p0 + w]
                nc.tensor.matmul(
                    xt_p[:w, j, 0:192],
                    lhsT=x_c,
                    rhs=identwt[:],
                    start=True,
                    stop=True,
                )
            gc = grp[0]
            # alternate evacuation between DVE and ACT so neither engine's
            # batch work starves the PE's PSUM recycling
            if gi % 2 == 0:
                nc.vector.tensor_copy(
                    xtr[:, gc : gc + gn, 0:128], xt_p[:, 0:gn, 0:128]
                )
                nc.scalar.copy(lgs[:, gc : gc + gn, :], xt_p[:, 0:gn, 128:192])
            else:
                nc.scalar.copy(xtr[:, gc : gc + gn, 0:128], xt_p[:, 0:gn, 0:128])
                nc.vector.tensor_copy(lgs[:, gc : gc + gn, :], xt_p[:, 0:gn, 128:192])
            # interleave the lagged sample's accumulation matmuls between
            # groups: short FWL acc matmuls fill the fused-matmul LDW bubbles
            if cpass is not None:
                emit_passC_chunks(cpass, gc, gc + gn)

        state[n] = (xs, xtr, et, sbt, lgs, x2t, slgt)

    def emit_scalars(n):
        xs, xtr, et, sbt, lgs, x2t, slgt = state[n]
        ss = smalls.tile([128, NCH], f32, tag="ss")
        zz = smalls.tile([128, NCH], f32, tag="zz")
        is_ = smalls.tile([128, NCH], f32, tag="is")
        tsc = smalls.tile([128, NCH], bf16, tag="tsc")

        # everything split per quarter-sample: no batch op is long enough to
        # stall the PSUM evacuation and let the PE's HAM throttle kick in
        qn = (NCH + 3) // 4
        halves = [(i * qn, min((i + 1) * qn, NCH)) for i in range(4)]
        lns = smalls.tile([128, NCH], f32, tag="lns")
        x2vf = x2t[:].rearrange("p (c d) -> p c d", c=NCH)
        for h0, h1 in halves:
            nc.scalar.activation(x2vf[:, h0:h1, :], xtr[:, h0:h1, 0:128], AF.Square)
            nc.vector.tensor_reduce(
                out=ss[:, h0:h1], in_=x2vf[:, h0:h1, :], axis=X_AX, op=ALU.add
            )
            # inv_s = exp(-0.5*ln(ss)); Ln+Exp live in one ACT table set
            nc.scalar.activation(lns[:, h0:h1], ss[:, h0:h1], AF.Ln)
            nc.scalar.activation(is_[:, h0:h1], lns[:, h0:h1], AF.Exp, scale=-0.5)
            # s = ss * inv_s = ||x_p||, into col 128 of each xtr chunk (the
            # acc matmul's rhs column turning sb into sum_sa)
            nc.gpsimd.tensor_tensor(
                out=xtr[:, h0:h1, 128],
                in0=ss[:, h0:h1],
                in1=is_[:, h0:h1],
                op=ALU.mult,
            )

        for h0, h1 in halves:
            slg = slgt[:, h0 * K : h1 * K].rearrange("p (c k) -> p c k", c=h1 - h0)
            nc.gpsimd.tensor_tensor(
                out=slg,
                in0=lgs[:, h0:h1, :],
                in1=bcast(is_[:, h0:h1], K),
                op=ALU.mult,
            )
            nc.scalar.activation(et[:, h0:h1, :], slg, AF.Exp)
            nc.vector.tensor_reduce(
                out=zz[:, h0:h1], in_=et[:, h0:h1, :], axis=X_AX, op=ALU.add
            )
            rr = smalls.tile([128, qn], f32, tag="rr")
            hw_ = h1 - h0
            nc.vector.reciprocal(rr[:, 0:hw_], zz[:, h0:h1])
            # t = inv_s / Z
            nc.gpsimd.tensor_tensor(
                out=tsc[:, h0:h1], in0=is_[:, h0:h1], in1=rr[:, 0:hw_], op=ALU.mult
            )
            # sb = e * t
            nc.gpsimd.tensor_tensor(
                out=sbt[:, h0:h1, :],
                in0=et[:, h0:h1, :],
                in1=bcast(tsc[:, h0:h1], K),
                op=ALU.mult,
            )

    cstate = {}  # open accumulation tiles for interleaved pass C

    def emit_passC_chunks(n, c0, c1):
        xs, xtr, et, sbt, lgs, x2t, slgt = state[n]
        if n not in cstate:
            acc_new = pp_acc.tile([K, 129], f32, tag="acc")
            cstate[n] = acc_new
        acc_p = cstate[n]
        for c in range(c0, min(c1, NCH)):
            p0, w = CHUNKS[c]
            nc.tensor.matmul(
                acc_p[:, :],
                lhsT=sbt[:w, c, :],
                rhs=xtr[:w, c, :],
                start=(c == 0),
                stop=(c == NCH - 1),
            )

    def finish_passC(n, agg_all, ssa_all):
        acc_p = cstate.pop(n)
        state.pop(n)
        # evacuate: agg = cols 0:128; sum_sa = col 128
        nc.vector.tensor_copy(agg_all[:, n, :], acc_p[:, 0:D])
        nc.scalar.copy(ssa_all[:, n : n + 1], acc_p[:, 128:129])

    def emit_passC(n, agg_all, ssa_all):
        emit_passC_chunks(n, 0, NCH)
        finish_passC(n, agg_all, ssa_all)

    # batched across all samples
    agg_all = tails.tile([K, NS, D], f32)
    ssa_all = tails.tile([K, NS], f32)

    def emit_tail(n0, n1):
        nn = n1 - n0
        agg_h = agg_all[:, n0:n1, :]
        ssa_h = ssa_all[:, n0:n1]
        vl = tails.tile([K, nn, D], f32, tag=f"t_vl{n0}")
        vsq = tails.tile([K, nn * D], f32, tag=f"t_vsq{n0}")
        q = tails.tile([K, nn], f32, tag=f"t_q{n0}")
        qm = tails.tile([K, nn], f32, tag=f"t_qm{n0}")
        isq = tails.tile([K, nn], f32, tag=f"t_isq{n0}")
        isq2 = tails.tile([K, nn], f32, tag=f"t_isq2{n0}")
        u = tails.tile([K, nn], f32, tag=f"t_u{n0}")
        gisr = tails.tile([1, nn], f32, tag=f"t_gisr{n0}")
        gb = tails.tile([K, nn], f32, tag=f"t_gb{n0}")
        sall = tails.tile([K, nn], f32, tag=f"t_s{n0}")
        vf = tails.tile([K, nn, D], f32, tag=f"t_vf{n0}")

        # vl = agg - ssa * cent
        nc.gpsimd.tensor_tensor(
            out=vl[:], in0=bcast(ssa_h, D), in1=mid_bcast(cent_s[:], nn), op=ALU.mult
        )
        nc.vector.tensor_tensor(out=vl[:], in0=agg_h, in1=vl[:], op=ALU.subtract)
        # q = rowsum(vl^2) per (k, n)
        vsqv = vsq[:].rearrange("k (n d) -> k n d", n=nn)
        nc.scalar.activation(vsqv, vl[:], AF.Square)
        nc.vector.tensor_reduce(out=q[:], in_=vsqv, axis=X_AX, op=ALU.add)
        nc.vector.tensor_scalar_max(qm[:], q[:], 1e-24)
        lq = tails.tile([K, nn], f32, tag=f"t_lq{n0}")
        nc.scalar.activation(lq[:], qm[:], AF.Ln)
        nc.scalar.activation(isq[:], lq[:], AF.Exp, scale=-0.5)
        # g = sum_k q_k * isq_k^2  (per sample)
        nc.vector.tensor_tensor(out=isq2[:], in0=isq[:], in1=isq[:], op=ALU.mult)
        nc.vector.tensor_tensor(out=u[:], in0=q[:], in1=isq2[:], op=ALU.mult)
        g_p = pp_tiny.tile([NS, 1], f32, tag="tiny")
        nc.tensor.matmul(
            g_p[:nn, :], lhsT=u[:], rhs=ones_col[:], start=True, stop=True
        )
        # gis = g^-0.5 -> transpose to a row -> broadcast over k partitions
        gm = tails.tile([nn, 1], f32, tag=f"t_gm{n0}")
        nc.vector.tensor_scalar_max(gm[:], g_p[:nn, :], 1e-24)
        gis = tails.tile([nn, 1], f32, tag=f"t_gis{n0}")
        lgm = tails.tile([nn, 1], f32, tag=f"t_lgm{n0}")
        nc.scalar.activation(lgm[:], gm[:], AF.Ln)
        nc.scalar.activation(gis[:], lgm[:], AF.Exp, scale=-0.5)
        gr_p = pp_tiny.tile([1, NS], f32, tag="tiny")
        nc.tensor.matmul(
            gr_p[:, :nn], lhsT=gis[:], rhs=ident[:nn, :nn], start=True, stop=True
        )
        nc.vector.tensor_copy(gisr[:], gr_p[:, :nn])
        gb_p = pp_tiny.tile([K, NS], f32, tag="tiny")
        nc.tensor.matmul(
            gb_p[:, :nn], lhsT=ones_row[:], rhs=gisr[:], start=True, stop=True
        )
        nc.vector.tensor_copy(gb[:], gb_p[:, :nn])
        # s = isq * gb; vf = vl * s
        nc.vector.tensor_tensor(out=sall[:], in0=isq[:], in1=gb[:], op=ALU.mult)
        nc.gpsimd.tensor_tensor(out=vf[:], in0=vl[:], in1=bcast(sall[:], D), op=ALU.mult)
        nc.sync.dma_start(
            out=out_ap.rearrange("n k d -> k n d")[:, n0:n1, :], in_=vf[:]
        )

    # emission order per round: pass A of sample n FIRST (so its PSUM-evac
    # copies sit ahead of batch reduces in the DVE/ACT queues), then the
    # scalar chain of n-1, then the acc matmuls of n-3.
    PIPE = 3
    for n in range(NS):
        emit_load_and_passA(n, cpass=(n - PIPE) if n >= PIPE else None)
        if n >= 1:
            emit_scalars(n - 1)
        if n >= PIPE:
            finish_passC(n - PIPE, agg_all, ssa_all)
            if n - PIPE == NS // 2 - 1:
                emit_tail(0, NS // 2)
    emit_passC(NS - PIPE, agg_all, ssa_all)
    emit_scalars(NS - 1)
    for n in range(NS - PIPE + 1, NS):
        emit_passC(n, agg_all, ssa_all)
    emit_tail(NS // 2, NS)


def kernel(x, conv_w, centroids):
    from concourse.bass_utils import run_bass_kernel_spmd

    if "nc" not in _CACHE:
        _CACHE["nc"] = _build_nc()
    nc = _CACHE["nc"]

    x = np.ascontiguousarray(np.asarray(x, dtype=np.float32).reshape(N, D, HW))
    wt = np.ascontiguousarray(np.asarray(conv_w, dtype=np.float32).T)
    cent = np.ascontiguousarray(np.asarray(centroids, dtype=np.float32))
    in_maps = [
        {"x": x[i * NS : (i + 1) * NS], "wt": wt, "cent": cent} for i in range(NCORES)
    ]
    res = run_bass_kernel_spmd(nc, in_maps, core_ids=list(range(NCORES))).results
    out = np.concatenate([r["out"].reshape(NS, K * D) for r in res], axis=0)
    return out


if __name__ == "__main__":
    rng = np.random.default_rng(0)
    xs = rng.standard_normal((N, D, 60, 80), dtype=np.float32)
    cw = (rng.standard_normal((K, D)) * 0.1).astype(np.float32)
    ct = rng.random((K, D), dtype=np.float32)
    o = kernel(x=xs, conv_w=cw, centroids=ct)
    print("kernel out", o.shape, o.dtype, np.abs(o).max())



# revision 5
# speedup vs baseline: 1.0137x; 1.0137x over previous
"""NetVLAD Trainium2 Bass kernel, v2 (all-bf16 matmul path).

Full inputs in, full output out. Data-parallel over batch N=64 across 8
NeuronCores (8 samples per core); conv weight and centroids replicated.

v2 structure (vs the f32 baseline):
  - x is converted to bf16 on the host: halves HBM traffic and makes every
    PE stream 1 cycle/row instead of 4 (fp32 is emitted as 2 half-rate MMs).
  - The per-chunk transpose uses tensor.transpose (transpose-mode) whose
    output dtype follows the input: bf16 lands in PSUM, so the big
    PSUM->SBUF evacuation runs in the DVE 2x packed mode.
  - Transpose and logits are separate matmuls (shared stationary x_c):
    xt goes to a bf16 PSUM pool (1 bank = 8 chunks), logits to an f32 pool
    (5 banks = one whole sample), evacuated to SBUF bf16 by ACT while the
    scalar chain of the previous sample runs.
  - slg/et/sbt use a [p, k, c] layout (c innermost) so the per-(p,c)
    broadcast multiply sbt = et * t runs in DVE 2x mode; slg runs on GPSIMD
    (mixed bf16/f32 broadcast, as in the baseline).
  - ss = sum_d x^2 uses square + a pairwise tree (2x mode) + one short 1x
    reduce instead of a full-width 1x tensor_reduce.
  - The tail's cross-partition sum uses gpsimd.partition_all_reduce, so no
    PSUM bank is needed for tiny matmuls (8 banks: 2 xt + 5 lg + 1 acc).

Pipelining: round n emits the scalar chain of sample n-1 first (its logits
evac frees the lg banks the round's own logits matmuls need), then the
transpose waves interleaved with the acc matmuls of sample n-3 and DVE
chain pieces, then the logits matmuls.
"""

import sys

if "/opt/trn_rl_repo" not in sys.path:
    sys.path.insert(0, "/opt/trn_rl_repo")

import numpy as np
from contextlib import ExitStack

N, D, HW, K = 64, 128, 4800, 64
NCORES = 8
NS = N // NCORES  # samples per core

CHUNKS = [(i * 128, min(128, HW - i * 128)) for i in range((HW + 127) // 128)]
NCH = len(CHUNKS)  # 38: 37 full + one 64-wide
XTS = 130  # xtr row stride (129 used; 130 keeps 4B alignment of chunk rows)

# waves of chunks sharing one xt PSUM bank
WAVES = [(w * 8, min(8, NCH - w * 8)) for w in range((NCH + 7) // 8)]  # 5 waves
NLG = len(WAVES)  # lg tiles per sample (5)

PIPE = 3  # acc matmuls of sample n-PIPE run during round n

_CACHE = {}


def _patch_act_tables():
    """Steer bacc's ACT table-set placement to the one set that covers
    every function we use (ln/exp/square/copy) so the kernel pays a single
    ACT_TABLE_LOAD instead of thrashing between per-anchor sets."""
    if _CACHE.get("act_patched"):
        return
    from concourse import bacc, mybir

    orig = bacc.get_activation_tables
    AF = mybir.ActivationFunctionType
    combo = "natural_log_exp_and_others"

    def patched(arch):
        t = {k: set(v) for k, v in orig(arch).items()}
        if combo in t:
            for name in t:
                if name != combo:
                    t[name] = t[name] - {AF.Ln, AF.Exp}
        return t

    bacc.get_activation_tables = patched
    _CACHE["act_patched"] = True


def _build_nc():
    import concourse.tile as tile
    from concourse import bacc, mybir

    _patch_act_tables()

    nc = bacc.Bacc(
        "TRN2",
        target_bir_lowering=False,
        debug=False,
        enable_asserts=False,
        num_devices=NCORES,
    )
    x_ap = nc.dram_tensor(
        "x", [NS, D, HW], mybir.dt.bfloat16, kind="ExternalInput"
    ).ap()
    wt_ap = nc.dram_tensor("wt", [D, K], mybir.dt.bfloat16, kind="ExternalInput").ap()
    cent_ap = nc.dram_tensor(
        "cent", [K, D], mybir.dt.float32, kind="ExternalInput"
    ).ap()
    out_ap = nc.dram_tensor(
        "out", [NS, K, D], mybir.dt.float32, kind="ExternalOutput"
    ).ap()

    with tile.TileContext(nc) as tc:
        with ExitStack() as ctx:
            _body(ctx, tc, out_ap, x_ap, wt_ap, cent_ap)
    nc.compile()
    return nc


def _body(ctx, tc, out_ap, x_ap, wt_ap, cent_ap):
    import concourse.bass as bass
    from concourse import bass_isa, masks, mybir

    nc = tc.nc
    f32 = mybir.dt.float32
    bf16 = mybir.dt.bfloat16
    AF = mybir.ActivationFunctionType
    ALU = mybir.AluOpType
    X_AX = mybir.AxisListType.X

    singles = ctx.enter_context(tc.tile_pool(name="singles", bufs=1))
    xspool = ctx.enter_context(tc.tile_pool(name="xspool", bufs=2))
    xtrpool = ctx.enter_context(tc.tile_pool(name="xtrpool", bufs=PIPE + 1))
    x2pool = ctx.enter_context(tc.tile_pool(name="x2pool", bufs=2))
    lgspool = ctx.enter_context(tc.tile_pool(name="lgspool", bufs=2))
    etpool = ctx.enter_context(tc.tile_pool(name="etpool", bufs=2))
    sbtpool = ctx.enter_context(tc.tile_pool(name="sbtpool", bufs=PIPE))
    smalls = ctx.enter_context(tc.tile_pool(name="smalls", bufs=2))
    tails = ctx.enter_context(tc.tile_pool(name="tails", bufs=1))
    pp_xt = ctx.enter_context(tc.tile_pool(name="pp_xt", bufs=2, space="PSUM"))
    pp_lg = ctx.enter_context(tc.tile_pool(name="pp_lg", bufs=NLG, space="PSUM"))
    pp_acc = ctx.enter_context(tc.tile_pool(name="pp_acc", bufs=1, space="PSUM"))

    def bcast(ap, n):
        # append a step-0 free dim: [..., n] broadcast view
        return bass.AP(tensor=ap.tensor, offset=ap.offset, ap=list(ap.ap) + [[0, n]])

    def mid_bcast(ap, n):
        # [p, f] -> [p, n, f] with step-0 middle dim
        return bass.AP(
            tensor=ap.tensor,
            offset=ap.offset,
            ap=[ap.ap[0], [0, n]] + list(ap.ap[1:]),
        )

    # constants
    identb = singles.tile([128, 128], bf16)
    masks.make_identity(nc, identb[:])
    wtb = singles.tile([D, K], bf16)
    nc.sync.dma_start(out=wtb[:], in_=wt_ap[:])
    cent_s = singles.tile([K, D], f32)
    nc.sync.dma_start(out=cent_s[:], in_=cent_ap[:])

    # per-sample live tiles
    state = {}
    cstate = {}
    evac_parity = [0]

    def emit_load(n):
        xs = xspool.tile([D, HW], bf16, tag="xs")
        nc.sync.dma_start(out=xs[:, 0 : HW // 2], in_=x_ap[n, :, 0 : HW // 2])
        nc.sync.dma_start(out=xs[:, HW // 2 :], in_=x_ap[n, :, HW // 2 :])
        return xs

    def emit_wave(n, w, chain_piece):
        """Transpose matmuls for wave w of sample n + evac + a DVE chain piece
        of sample n-1 queued behind the evac."""
        xs, st = state[n]
        c0, wn = WAVES[w]
        xt_p = pp_xt.tile([128, 8, 128], bf16, tag="xt")
        for j in range(wn):
            p0, cw = CHUNKS[c0 + j]
            nc.tensor.transpose(
                xt_p[:cw, j, :], xs[:, p0 : p0 + cw], identb[:]
            )
        xtr = st["xtr"]
        dst = xtr[:, c0 : c0 + wn, 0:128]
        src = xt_p[:, 0:wn, :]
        if evac_parity[0] % 2 == 0:
            nc.vector.tensor_copy(dst, src)
        else:
            nc.scalar.copy(dst, src)
        evac_parity[0] += 1
        if chain_piece is not None:
            chain_piece()

    def emit_lg_mms(n):
        xs, st = state[n]
        lgp = []
        for t in range(NLG):
            c0, wn = WAVES[t]
            lg_p = pp_lg.tile([128, 8, K], f32, tag="lg")
            for j in range(wn):
                p0, cw = CHUNKS[c0 + j]
                nc.tensor.matmul(
                    lg_p[:cw, j, :],
                    lhsT=xs[:, p0 : p0 + cw],
                    rhs=wtb[:],
                    start=True,
                    stop=True,
                )
            lgp.append(lg_p)
        st["lgp"] = lgp

    def chain_evac_lg(n):
        """ACT: evacuate the sample's logits PSUM banks to SBUF bf16 in
        [p, k, c] layout (c innermost). Frees lg banks for the next sample."""
        xs, st = state[n]
        lgs = lgspool.tile([128, K, NCH], bf16, tag="lgs")
        st["lgs"] = lgs
        for t, lg_p in enumerate(st["lgp"]):
            c0, wn = WAVES[t]
            # out view [p][c (step 1), wn][k (step NCH), K]
            dstv = bass.AP(
                tensor=lgs.tensor,
                offset=lgs.offset + c0,
                ap=[lgs.ap[0], [1, wn], [NCH, K]],
            )
            nc.scalar.copy(dstv, lg_p[:, 0:wn, :])
        st.pop("lgp")

    def make_chain_pieces(n):
        """DVE-side ss chain of sample n, split into pieces to interleave
        with the next sample's evac waves. Returns list of callables."""
        xs, st = state[n]
        xtr = st["xtr"]
        x2 = x2pool.tile([128, NCH, 128], bf16, tag="x2")
        t64 = x2pool.tile([128, NCH, 64], bf16, tag="t64")
        t32 = x2pool.tile([128, NCH, 32], bf16, tag="t32")
        t16 = x2pool.tile([128, NCH, 16], bf16, tag="t16")
        ss = smalls.tile([128, NCH], f32, tag="ss")
        lns = smalls.tile([128, NCH], f32, tag="lns")
        is_ = smalls.tile([128, NCH], f32, tag="is")
        st["ss"], st["is_"] = ss, is_

        HA = NCH // 2  # 19
        halves = [(0, HA), (HA, NCH)]

        def sq_half(h, eng):
            h0, h1 = halves[h]
            xv = xtr[:, h0:h1, 0:128]
            if eng == "v":
                nc.vector.tensor_tensor(
                    out=x2[:, h0:h1, :], in0=xv, in1=xv, op=ALU.mult
                )
            else:
                nc.scalar.activation(x2[:, h0:h1, :], xv, AF.Square)

        def tree_half(h):
            h0, h1 = halves[h]
            nc.vector.tensor_tensor(
                out=t64[:, h0:h1, :],
                in0=x2[:, h0:h1, 0:64],
                in1=x2[:, h0:h1, 64:128],
                op=ALU.add,
            )
            nc.vector.tensor_tensor(
                out=t32[:, h0:h1, :],
                in0=t64[:, h0:h1, 0:32],
                in1=t64[:, h0:h1, 32:64],
                op=ALU.add,
            )
            nc.vector.tensor_tensor(
                out=t16[:, h0:h1, :],
                in0=t32[:, h0:h1, 0:16],
                in1=t32[:, h0:h1, 16:32],
                op=ALU.add,
            )

        def red_half(h):
            h0, h1 = halves[h]
            nc.vector.tensor_reduce(
                out=ss[:, h0:h1], in_=t16[:, h0:h1, :], axis=X_AX, op=ALU.add
            )

        def rsqrt_all():
            # inv_s = exp(-0.5*ln(ss)); Ln+Exp live in one ACT table set
            nc.scalar.activation(lns[:], ss[:], AF.Ln)
            nc.scalar.activation(is_[:], lns[:], AF.Exp, scale=-0.5)
            # s-col: xtr[:, c, 128] = ss * inv_s = ||x_p||
            nc.gpsimd.tensor_tensor(
                out=xtr[:, :, 128], in0=ss[:], in1=is_[:], op=ALU.mult
            )

        return [
            lambda: sq_half(0, "v"),
            lambda: (sq_half(1, "s"), tree_half(0)),
            lambda: (red_half(0), tree_half(1)),
            lambda: (red_half(1), rsqrt_all()),
        ]

    def chain_softmax(n):
        """slg (gpsimd), exp (ACT), zz (gp L1 + DVE), sbt (DVE 2x)."""
        xs, st = state[n]
        lgs, is_ = st["lgs"], st["is_"]
        slg = lgspool.tile([128, K, NCH], bf16, tag="slg")
        et = etpool.tile([128, K, NCH], bf16, tag="et")
        zt = etpool.tile([128, K // 2, NCH], bf16, tag="zt")
        zz = smalls.tile([128, NCH], f32, tag="zz")
        rr = smalls.tile([128, NCH], f32, tag="rr")
        tsc = smalls.tile([128, NCH], bf16, tag="tsc")
        sbt = sbtpool.tile([128, K, NCH], bf16, tag="sbt")
        st["sbt"] = sbt

        nc.gpsimd.tensor_tensor(
            out=slg[:], in0=lgs[:], in1=mid_bcast(is_[:], K), op=ALU.mult
        )
        nc.scalar.activation(et[:], slg[:], AF.Exp)
        # zz = sum_k et: one gpsimd pairwise level, then a strided 1x reduce
        nc.gpsimd.tensor_tensor(
            out=zt[:], in0=et[:, 0 : K // 2, :], in1=et[:, K // 2 : K, :], op=ALU.add
        )
        ztv = bass.AP(
            tensor=zt.tensor,
            offset=zt.offset,
            ap=[zt.ap[0], [1, NCH], [NCH, K // 2]],
        )
        nc.vector.tensor_reduce(out=zz[:], in_=ztv, axis=X_AX, op=ALU.add)
        nc.vector.reciprocal(rr[:], zz[:])
        # t = inv_s / Z (bf16 so the sbt broadcast multiply gets 2x mode)
        nc.gpsimd.tensor_tensor(out=tsc[:], in0=is_[:], in1=rr[:], op=ALU.mult)
        nc.vector.tensor_tensor(
            out=sbt[:], in0=et[:], in1=mid_bcast(tsc[:], K), op=ALU.mult
        )

    def emit_acc_chunks(n, c0, c1):
        xs, st = state[n]
        xtr, sbt = st["xtr"], st["sbt"]
        if n not in cstate:
            cstate[n] = pp_acc.tile([K, 129], f32, tag="acc", name="acc")
        acc_p = cstate[n]
        for c in range(c0, min(c1, NCH)):
            p0, cw = CHUNKS[c]
            lhsv = bass.AP(
                tensor=sbt.tensor,
                offset=sbt.offset + c,
                ap=[[sbt.ap[0][0], cw], [NCH, K]],
            )
            nc.tensor.matmul(
                acc_p[:, :],
                lhsT=lhsv,
                rhs=xtr[:cw, c, 0:129],
                start=(c == 0),
                stop=(c == NCH - 1),
            )

    # batched across all samples
    agg_all = tails.tile([K, NS, D], f32)
    ssa_all = tails.tile([K, NS], f32)

    def finish_acc(n):
        acc_p = cstate.pop(n)
        nc.vector.tensor_copy(agg_all[:, n, :], acc_p[:, 0:D])
        nc.scalar.copy(ssa_all[:, n : n + 1], acc_p[:, 128:129])
        # state[n] no longer needed
        state.pop(n)

    def emit_tail(n0, n1):
        nn = n1 - n0
        agg_h = agg_all[:, n0:n1, :]
        ssa_h = ssa_all[:, n0:n1]
        vl = tails.tile([K, nn, D], f32, tag=f"t_vl{n0}")
        vsq = tails.tile([K, nn, D], f32, tag=f"t_vsq{n0}")
        q = tails.tile([K, nn], f32, tag=f"t_q{n0}")
        qm = tails.tile([K, nn], f32, tag=f"t_qm{n0}")
        lq = tails.tile([K, nn], f32, tag=f"t_lq{n0}")
        isq = tails.tile([K, nn], f32, tag=f"t_isq{n0}")
        isq2 = tails.tile([K, nn], f32, tag=f"t_isq2{n0}")
        u = tails.tile([K, nn], f32, tag=f"t_u{n0}")
        g = tails.tile([K, nn], f32, tag=f"t_g{n0}")
        gm = tails.tile([K, nn], f32, tag=f"t_gm{n0}")
        lgm = tails.tile([K, nn], f32, tag=f"t_lgm{n0}")
        gis = tails.tile([K, nn], f32, tag=f"t_gis{n0}")
        sall = tails.tile([K, nn], f32, tag=f"t_s{n0}")
        vf = tails.tile([K, nn, D], f32, tag=f"t_vf{n0}")

        # vl = agg - ssa * cent
        nc.gpsimd.tensor_tensor(
            out=vl[:], in0=bcast(ssa_h, D), in1=mid_bcast(cent_s[:], nn), op=ALU.mult
        )
        nc.vector.tensor_tensor(out=vl[:], in0=agg_h, in1=vl[:], op=ALU.subtract)
        # q = rowsum(vl^2) per (k, n)
        nc.scalar.activation(vsq[:], vl[:], AF.Square)
        nc.vector.tensor_reduce(out=q[:], in_=vsq[:], axis=X_AX, op=ALU.add)
        nc.vector.tensor_scalar_max(qm[:], q[:], 1e-24)
        nc.scalar.activation(lq[:], qm[:], AF.Ln)
        nc.scalar.activation(isq[:], lq[:], AF.Exp, scale=-0.5)
        # g[n] = sum_k q_k * isq_k^2, broadcast to all partitions
        nc.vector.tensor_tensor(out=isq2[:], in0=isq[:], in1=isq[:], op=ALU.mult)
        nc.vector.tensor_tensor(out=u[:], in0=q[:], in1=isq2[:], op=ALU.mult)
        nc.gpsimd.partition_all_reduce(
            g[:], u[:], channels=K, reduce_op=bass_isa.ReduceOp.add
        )
        nc.vector.tensor_scalar_max(gm[:], g[:], 1e-24)
        nc.scalar.activation(lgm[:], gm[:], AF.Ln)
        nc.scalar.activation(gis[:], lgm[:], AF.Exp, scale=-0.5)
        # s = isq * gis; vf = vl * s
        nc.vector.tensor_tensor(out=sall[:], in0=isq[:], in1=gis[:], op=ALU.mult)
        nc.gpsimd.tensor_tensor(
            out=vf[:], in0=vl[:], in1=bcast(sall[:], D), op=ALU.mult
        )
        nc.sync.dma_start(
            out=out_ap.rearrange("n k d -> k n d")[:, n0:n1, :], in_=vf[:]
        )

    # ---- main schedule ----
    # preload xs(0)
    xs0 = emit_load(0)
    state[0] = (xs0, {})
    pieces_prev = None  # DVE ss-chain pieces of sample n-1

    for n in range(NS):
        # prefetch next sample's x
        if n + 1 < NS:
            state[n + 1] = (emit_load(n + 1), {})
        # allocate this round's xtr
        state[n][1]["xtr"] = xtrpool.tile(
            [128, NCH, XTS], bf16, tag="xtr", name="xtr"
        )

        # chain part 1 of sample n-1: lg evac first (frees lg banks), then
        # the softmax block (gpsimd/ACT heavy, DVE-light)
        if n >= 1:
            chain_evac_lg(n - 1)
            pieces_prev = make_chain_pieces(n - 1)

        # transpose waves + interleaved DVE chain pieces + acc of n-PIPE
        acc_per_wave = (NCH + len(WAVES) - 1) // len(WAVES)
        for w in range(len(WAVES)):
            piece = None
            if pieces_prev is not None and w < len(pieces_prev):
                piece = pieces_prev[w]
            emit_wave(n, w, piece)
            if n >= PIPE:
                emit_acc_chunks(
                    n - PIPE, w * acc_per_wave, (w + 1) * acc_per_wave
                )
        pieces_prev = None

        # logits matmuls (lg banks freed by chain_evac_lg above)
        emit_lg_mms(n)

        # softmax tail of sample n-1 (needs is_ from the pieces above)
        if n >= 1:
            chain_softmax(n - 1)

        if n >= PIPE:
            finish_acc(n - PIPE)
            if n - PIPE == NS // 2 - 1:
                emit_tail(0, NS // 2)

    # drain: chain of the last sample, remaining accs
    chain_evac_lg(NS - 1)
    for piece in make_chain_pieces(NS - 1):
        piece()
    chain_softmax(NS - 1)
    for n in range(NS - PIPE, NS):
        emit_acc_chunks(n, 0, NCH)
        finish_acc(n)
    emit_tail(NS // 2, NS)


def kernel(x, conv_w, centroids):
    import ml_dtypes
    from concourse.bass_utils import run_bass_kernel_spmd

    if "nc" not in _CACHE:
        _CACHE["nc"] = _build_nc()
    nc = _CACHE["nc"]

    x = np.ascontiguousarray(
        np.asarray(x, dtype=np.float32).reshape(N, D, HW).astype(ml_dtypes.bfloat16)
    )
    wt = np.ascontiguousarray(
        np.asarray(conv_w, dtype=np.float32).T.astype(ml_dtypes.bfloat16)
    )
    cent = np.ascontiguousarray(np.asarray(centroids, dtype=np.float32))
    in_maps = [
        {"x": x[i * NS : (i + 1) * NS], "wt": wt, "cent": cent} for i in range(NCORES)
    ]
    res = run_bass_kernel_spmd(nc, in_maps, core_ids=list(range(NCORES))).results
    out = np.concatenate([r["out"].reshape(NS, K * D) for r in res], axis=0)
    return out


if __name__ == "__main__":
    rng = np.random.default_rng(0)
    xs = rng.standard_normal((N, D, 60, 80), dtype=np.float32)
    cw = (rng.standard_normal((K, D)) * 0.1).astype(np.float32)
    ct = rng.random((K, D), dtype=np.float32)
    o = kernel(x=xs, conv_w=cw, centroids=ct)
    print("kernel out", o.shape, o.dtype, np.abs(o).max())


# revision 6
# speedup vs baseline: 1.3863x; 1.3676x over previous
"""NetVLAD Trainium2 Bass kernel, v3 (bf16 matmuls, fused slg, col-tiled acc).

Full inputs in, full output out. Data-parallel over batch N=64 across 8
NeuronCores (8 samples per core); conv weight and centroids replicated.

Structure (per core, 8 samples, 38 pixel-chunks of 128 per sample):
  - x arrives bf16 (host-converted): half the HBM traffic, and every PE
    stream runs at 1 cycle/row (fp32 would be 4).
  - Per chunk: transpose-mode matmul writes x_c^T to a *bf16* PSUM bank
    (8 chunks/bank), evacuated to SBUF by DVE in the 2x packed mode /
    ACT; a second matmul with the same stationary x_c produces logits
    into an f32 PSUM pool large enough for a whole sample (5 banks).
  - slg = logits * inv_s is computed by DVE directly from logits PSUM
    (per-bank ops, step-0 broadcast of inv_s), which both evacuates and
    scales in one 1x pass and releases the logits banks.
  - ss = sum_d x^2: ACT squares, GPSIMD does one pairwise fold (128->64),
    DVE reduces the rest. inv_s = exp(-0.5 ln ss) on ACT.
  - softmax: exp on ACT, Z-reduce + reciprocal on DVE, t = inv_s/Z and
    sbt = et * t broadcasts on GPSIMD.
  - acc matmuls are column-tiled: even chunks accumulate into PSUM
    partitions 0:64, odd chunks into 64:128, running pairwise-concurrent
    in the PE array; the two halves are summed during evacuation.
  - Tail (VLAD normalizations) uses gpsimd.partition_all_reduce for the
    global norm, so no PSUM bank is needed for tiny matmuls.
  PSUM budget: 2 (xt bf16) + 5 (logits f32) + 1 (acc) = 8 banks.
"""

import sys

if "/opt/trn_rl_repo" not in sys.path:
    sys.path.insert(0, "/opt/trn_rl_repo")

import numpy as np
from contextlib import ExitStack

N, D, HW, K = 64, 128, 4800, 64
NCORES = 8
NS = N // NCORES  # samples per core

CHUNKS = [(i * 128, min(128, HW - i * 128)) for i in range((HW + 127) // 128)]
NCH = len(CHUNKS)  # 38: 37 full + one 64-wide
XTS = 130  # xtr row stride (129 used; 130 keeps 4B alignment of chunk rows)

# waves of chunks sharing one xt PSUM bank / one lg PSUM bank
WAVES = [(w * 8, min(8, NCH - w * 8)) for w in range((NCH + 7) // 8)]  # 5 waves
NLG = len(WAVES)

PIPE = 3  # acc matmuls of sample n-PIPE run during round n

_CACHE = {}


def _patch_act_tables():
    """Steer bacc's ACT table-set placement so ln/exp (and the cheap
    square/copy fillers) live in one set: a single ACT_TABLE_LOAD."""
    if _CACHE.get("act_patched"):
        return
    from concourse import bacc, mybir

    orig = bacc.get_activation_tables
    AF = mybir.ActivationFunctionType
    combo = "natural_log_exp_and_others"

    def patched(arch):
        t = {k: set(v) for k, v in orig(arch).items()}
        if combo in t:
            for name in t:
                if name != combo:
                    t[name] = t[name] - {AF.Ln, AF.Exp}
        return t

    bacc.get_activation_tables = patched
    _CACHE["act_patched"] = True


def _build_nc():
    import concourse.tile as tile
    from concourse import bacc, mybir

    _patch_act_tables()

    nc = bacc.Bacc(
        "TRN2",
        target_bir_lowering=False,
        debug=False,
        enable_asserts=False,
        num_devices=NCORES,
    )
    x_ap = nc.dram_tensor(
        "x", [NS, D, HW], mybir.dt.bfloat16, kind="ExternalInput"
    ).ap()
    wt_ap = nc.dram_tensor("wt", [D, K], mybir.dt.bfloat16, kind="ExternalInput").ap()
    cent_ap = nc.dram_tensor(
        "cent", [K, D], mybir.dt.float32, kind="ExternalInput"
    ).ap()
    out_ap = nc.dram_tensor(
        "out", [NS, K, D], mybir.dt.float32, kind="ExternalOutput"
    ).ap()

    with tile.TileContext(nc) as tc:
        with ExitStack() as ctx:
            _body(ctx, tc, out_ap, x_ap, wt_ap, cent_ap)
    nc.compile()
    return nc


def _body(ctx, tc, out_ap, x_ap, wt_ap, cent_ap):
    import concourse.bass as bass
    from concourse import bass_isa, masks, mybir

    nc = tc.nc
    f32 = mybir.dt.float32
    bf16 = mybir.dt.bfloat16
    AF = mybir.ActivationFunctionType
    ALU = mybir.AluOpType
    X_AX = mybir.AxisListType.X

    singles = ctx.enter_context(tc.tile_pool(name="singles", bufs=1))
    xspool = ctx.enter_context(tc.tile_pool(name="xspool", bufs=2))
    xtrpool = ctx.enter_context(tc.tile_pool(name="xtrpool", bufs=PIPE + 1))
    x2pool = ctx.enter_context(tc.tile_pool(name="x2pool", bufs=2))
    slgpool = ctx.enter_context(tc.tile_pool(name="slgpool", bufs=2))
    etpool = ctx.enter_context(tc.tile_pool(name="etpool", bufs=2))
    sbtpool = ctx.enter_context(tc.tile_pool(name="sbtpool", bufs=PIPE))
    smalls = ctx.enter_context(tc.tile_pool(name="smalls", bufs=2))
    tails = ctx.enter_context(tc.tile_pool(name="tails", bufs=1))
    pp_xt = ctx.enter_context(tc.tile_pool(name="pp_xt", bufs=2, space="PSUM"))
    pp_lg = ctx.enter_context(tc.tile_pool(name="pp_lg", bufs=NLG, space="PSUM"))
    pp_acc = ctx.enter_context(tc.tile_pool(name="pp_acc", bufs=1, space="PSUM"))

    def bcast(ap, n):
        # append a step-0 free dim: [..., n] broadcast view
        return bass.AP(tensor=ap.tensor, offset=ap.offset, ap=list(ap.ap) + [[0, n]])

    def mid_bcast(ap, n):
        # [p, f] -> [p, n, f] with step-0 middle dim
        return bass.AP(
            tensor=ap.tensor,
            offset=ap.offset,
            ap=[ap.ap[0], [0, n]] + list(ap.ap[1:]),
        )

    # constants
    identb = singles.tile([128, 128], bf16)
    masks.make_identity(nc, identb[:])
    wtb = singles.tile([D, K], bf16)
    nc.sync.dma_start(out=wtb[:], in_=wt_ap[:])
    cent_s = singles.tile([K, D], f32)
    nc.sync.dma_start(out=cent_s[:], in_=cent_ap[:])

    state = {}  # n -> dict of live tiles
    cstate = {}  # n -> acc psum tile

    def emit_load(n):
        xs = xspool.tile([D, HW], bf16, tag="xs", name="xs")
        nc.sync.dma_start(out=xs[:, 0 : HW // 2], in_=x_ap[n, :, 0 : HW // 2])
        nc.sync.dma_start(out=xs[:, HW // 2 :], in_=x_ap[n, :, HW // 2 :])
        return xs

    # ---- softmax block of sample m (runs at round m+1 start) ----
    def emit_softmax(m):
        st = state[m]
        is_ = st["is_"]
        slg = slgpool.tile([128, NCH, K], bf16, tag="slg", name="slg")
        et = etpool.tile([128, NCH, K], bf16, tag="et", name="et")
        zz = smalls.tile([128, NCH], f32, tag="zz", name="zz")
        rr = smalls.tile([128, NCH], f32, tag="rr", name="rr")
        tsc = smalls.tile([128, NCH], bf16, tag="tsc", name="tsc")
        sbt = sbtpool.tile([128, NCH, K], bf16, tag="sbt", name="sbt")
        st["sbt"] = sbt

        # slg = lg * inv_s straight out of PSUM (evac + scale in one pass);
        # releases lg bank t for the new round's logits matmuls
        for t, lg_p in enumerate(st["lgp"]):
            c0, wn = WAVES[t]
            nc.vector.tensor_tensor(
                out=slg[:, c0 : c0 + wn, :],
                in0=lg_p[:, 0:wn, :],
                in1=bcast(is_[:, c0 : c0 + wn], K),
                op=ALU.mult,
            )
        st.pop("lgp")
        HA = 2 * (NCH // 4)  # 18, wave-tile friendly split not needed here
        for h0, h1 in ((0, HA), (HA, NCH)):
            nc.scalar.activation(et[:, h0:h1, :], slg[:, h0:h1, :], AF.Exp)
            nc.vector.tensor_reduce(
                out=zz[:, h0:h1], in_=et[:, h0:h1, :], axis=X_AX, op=ALU.add
            )
        nc.vector.reciprocal(rr[:], zz[:])
        # t = inv_s / Z (bf16)
        nc.gpsimd.tensor_tensor(out=tsc[:], in0=is_[:], in1=rr[:], op=ALU.mult)
        for h0, h1 in ((0, HA), (HA, NCH)):
            nc.gpsimd.tensor_tensor(
                out=sbt[:, h0:h1, :],
                in0=et[:, h0:h1, :],
                in1=bcast(tsc[:, h0:h1], K),
                op=ALU.mult,
            )

    # ---- transpose waves of sample n ----
    def emit_wave(n, w):
        st = state[n]
        xs, xtr = st["xs"], st["xtr"]
        c0, wn = WAVES[w]
        xt_p = pp_xt.tile([128, 8, 128], bf16, tag="xt", name="xt")
        for j in range(wn):
            p0, cw = CHUNKS[c0 + j]
            nc.tensor.transpose(xt_p[:cw, j, :], xs[:, p0 : p0 + cw], identb[:])
        dst = xtr[:, c0 : c0 + wn, 0:128]
        src = xt_p[:, 0:wn, :]
        if w % 2 == 0:
            nc.vector.tensor_copy(dst, src)
        else:
            nc.scalar.copy(dst, src)

    def emit_lg_mms(n):
        st = state[n]
        xs = st["xs"]
        lgp = []
        for t in range(NLG):
            c0, wn = WAVES[t]
            lg_p = pp_lg.tile([128, 8, K], f32, tag="lg", name="lg")
            for j in range(wn):
                p0, cw = CHUNKS[c0 + j]
                nc.tensor.matmul(
                    lg_p[:cw, j, :],
                    lhsT=xs[:, p0 : p0 + cw],
                    rhs=wtb[:],
                    start=True,
                    stop=True,
                )
            lgp.append(lg_p)
        st["lgp"] = lgp

    # ---- ss chain of sample n (round tail): is_ ready for round n+1 ----
    def emit_ss_chain(n):
        st = state[n]
        xtr = st["xtr"]
        x2 = x2pool.tile([128, NCH, 128], bf16, tag="x2", name="x2")
        t64 = x2pool.tile([128, NCH, 64], bf16, tag="t64", name="t64")
        ss = smalls.tile([128, NCH], f32, tag="ss", name="ss")
        lns = smalls.tile([128, NCH], f32, tag="lns", name="lns")
        is_ = smalls.tile([128, NCH], f32, tag="is", name="is")
        st["is_"] = is_

        HA = NCH // 2  # 19
        for h0, h1 in ((0, HA), (HA, NCH)):
            nc.scalar.activation(
                x2[:, h0:h1, :], xtr[:, h0:h1, 0:128], AF.Square
            )
            nc.gpsimd.tensor_tensor(
                out=t64[:, h0:h1, :],
                in0=x2[:, h0:h1, 0:64],
                in1=x2[:, h0:h1, 64:128],
                op=ALU.add,
            )
            nc.vector.tensor_reduce(
                out=ss[:, h0:h1], in_=t64[:, h0:h1, :], axis=X_AX, op=ALU.add
            )
        # inv_s = exp(-0.5*ln(ss)); Ln+Exp share one ACT table set
        nc.scalar.activation(lns[:], ss[:], AF.Ln)
        nc.scalar.activation(is_[:], lns[:], AF.Exp, scale=-0.5)
        # s-col: xtr[:, c, 128] = ss * inv_s = ||x_p||
        nc.gpsimd.tensor_tensor(
            out=xtr[:, :, 128], in0=ss[:], in1=is_[:], op=ALU.mult
        )

    # ---- col-tiled acc matmuls of sample m ----
    def emit_acc_chunks(m, c0, c1):
        st = state[m]
        xtr, sbt = st["xtr"], st["sbt"]
        if m not in cstate:
            cstate[m] = pp_acc.tile([128, 129], f32, tag="acc", name="acc")
        acc_p = cstate[m]
        for c in range(c0, min(c1, NCH)):
            p0, cw = CHUNKS[c]
            half = c % 2
            nc.tensor.matmul(
                acc_p[64 * half : 64 * half + 64, :],
                lhsT=sbt[:cw, c, :],
                rhs=xtr[:cw, c, 0:129],
                start=(c < 2),
                stop=(c >= NCH - 2),
                tile_position=(0, 64 * half),
            )

    agg_all = tails.tile([K, NS, D], f32)
    ssa_all = tails.tile([K, NS], f32)

    def finish_acc(m):
        acc_p = cstate.pop(m)
        # agg = even-chunk half + odd-chunk half; same for sum_sa column
        nc.scalar.copy(agg_all[:, m, :], acc_p[0:64, 0:D])
        nc.vector.tensor_tensor(
            out=agg_all[:, m, :],
            in0=agg_all[:, m, :],
            in1=acc_p[64:128, 0:D],
            op=ALU.add,
        )
        nc.scalar.copy(ssa_all[:, m : m + 1], acc_p[0:64, 128:129])
        nc.vector.tensor_tensor(
            out=ssa_all[:, m : m + 1],
            in0=ssa_all[:, m : m + 1],
            in1=acc_p[64:128, 128:129],
            op=ALU.add,
        )
        state.pop(m)

    def emit_tail(n0, n1):
        nn = n1 - n0
        agg_h = agg_all[:, n0:n1, :]
        ssa_h = ssa_all[:, n0:n1]
        vl = tails.tile([K, nn, D], f32, tag=f"t_vl{n0}", name="vl")
        vsq = tails.tile([K, nn, D], f32, tag=f"t_vsq{n0}", name="vsq")
        q = tails.tile([K, nn], f32, tag=f"t_q{n0}", name="q")
        qm = tails.tile([K, nn], f32, tag=f"t_qm{n0}", name="qm")
        lq = tails.tile([K, nn], f32, tag=f"t_lq{n0}", name="lq")
        isq = tails.tile([K, nn], f32, tag=f"t_isq{n0}", name="isq")
        isq2 = tails.tile([K, nn], f32, tag=f"t_isq2{n0}", name="isq2")
        u = tails.tile([K, nn], f32, tag=f"t_u{n0}", name="u")
        g = tails.tile([K, nn], f32, tag=f"t_g{n0}", name="g")
        gm = tails.tile([K, nn], f32, tag=f"t_gm{n0}", name="gm")
        lgm = tails.tile([K, nn], f32, tag=f"t_lgm{n0}", name="lgm")
        gis = tails.tile([K, nn], f32, tag=f"t_gis{n0}", name="gis")
        sall = tails.tile([K, nn], f32, tag=f"t_s{n0}", name="sall")
        vf = tails.tile([K, nn, D], f32, tag=f"t_vf{n0}", name="vf")

        # vl = agg - ssa * cent
        nc.gpsimd.tensor_tensor(
            out=vl[:], in0=bcast(ssa_h, D), in1=mid_bcast(cent_s[:], nn), op=ALU.mult
        )
        nc.vector.tensor_tensor(out=vl[:], in0=agg_h, in1=vl[:], op=ALU.subtract)
        # q = rowsum(vl^2) per (k, n)
        nc.scalar.activation(vsq[:], vl[:], AF.Square)
        nc.vector.tensor_reduce(out=q[:], in_=vsq[:], axis=X_AX, op=ALU.add)
        nc.vector.tensor_scalar_max(qm[:], q[:], 1e-24)
        nc.scalar.activation(lq[:], qm[:], AF.Ln)
        nc.scalar.activation(isq[:], lq[:], AF.Exp, scale=-0.5)
        # g[n] = sum_k q_k * isq_k^2, all-reduced across partitions
        nc.vector.tensor_tensor(out=isq2[:], in0=isq[:], in1=isq[:], op=ALU.mult)
        nc.vector.tensor_tensor(out=u[:], in0=q[:], in1=isq2[:], op=ALU.mult)
        nc.gpsimd.partition_all_reduce(
            g[:], u[:], channels=K, reduce_op=bass_isa.ReduceOp.add
        )
        nc.vector.tensor_scalar_max(gm[:], g[:], 1e-24)
        nc.scalar.activation(lgm[:], gm[:], AF.Ln)
        nc.scalar.activation(gis[:], lgm[:], AF.Exp, scale=-0.5)
        # s = isq * gis; vf = vl * s
        nc.vector.tensor_tensor(out=sall[:], in0=isq[:], in1=gis[:], op=ALU.mult)
        nc.gpsimd.tensor_tensor(
            out=vf[:], in0=vl[:], in1=bcast(sall[:], D), op=ALU.mult
        )
        nc.sync.dma_start(
            out=out_ap.rearrange("n k d -> k n d")[:, n0:n1, :], in_=vf[:]
        )

    # ---- main schedule ----
    state[0] = {"xs": emit_load(0)}
    acc_per_wave = (NCH + len(WAVES) - 1) // len(WAVES)

    for n in range(NS):
        if n + 1 < NS:
            state[n + 1] = {"xs": emit_load(n + 1)}
        state[n]["xtr"] = xtrpool.tile(
            [128, NCH, XTS], bf16, tag="xtr", name="xtr"
        )
        # softmax block of n-1: frees the lg banks before this round's
        # logits matmuls are reached by the PE
        if n >= 1:
            emit_softmax(n - 1)
        # transpose waves + acc matmuls of n-PIPE interleaved on the PE
        for w in range(len(WAVES)):
            emit_wave(n, w)
            if n >= PIPE:
                emit_acc_chunks(n - PIPE, w * acc_per_wave, (w + 1) * acc_per_wave)
        emit_lg_mms(n)
        if n >= PIPE:
            finish_acc(n - PIPE)
            if n - PIPE == NS // 2 - 1:
                emit_tail(0, NS // 2)
        # ss chain of n (is_ ready for round n+1's softmax block)
        emit_ss_chain(n)

    # drain
    emit_softmax(NS - 1)
    for m in range(NS - PIPE, NS):
        emit_acc_chunks(m, 0, NCH)
        finish_acc(m)
    emit_tail(NS // 2, NS)


def kernel(x, conv_w, centroids):
    import ml_dtypes
    from concourse.bass_utils import run_bass_kernel_spmd

    if "nc" not in _CACHE:
        _CACHE["nc"] = _build_nc()
    nc = _CACHE["nc"]

    x = np.ascontiguousarray(
        np.asarray(x, dtype=np.float32).reshape(N, D, HW).astype(ml_dtypes.bfloat16)
    )
    wt = np.ascontiguousarray(
        np.asarray(conv_w, dtype=np.float32).T.astype(ml_dtypes.bfloat16)
    )
    cent = np.ascontiguousarray(np.asarray(centroids, dtype=np.float32))
    in_maps = [
        {"x": x[i * NS : (i + 1) * NS], "wt": wt, "cent": cent} for i in range(NCORES)
    ]
    res = run_bass_kernel_spmd(nc, in_maps, core_ids=list(range(NCORES))).results
    out = np.concatenate([r["out"].reshape(NS, K * D) for r in res], axis=0)
    return out


if __name__ == "__main__":
    rng = np.random.default_rng(0)
    xs = rng.standard_normal((N, D, 60, 80), dtype=np.float32)
    cw = (rng.standard_normal((K, D)) * 0.1).astype(np.float32)
    ct = rng.random((K, D), dtype=np.float32)
    o = kernel(x=xs, conv_w=cw, centroids=ct)
    print("kernel out", o.shape, o.dtype, np.abs(o).max())


# revision 12
# speedup vs baseline: 1.4424x; 1.0405x over previous
"""NetVLAD Trainium2 Bass kernel, v3 (bf16 matmuls, fused slg, col-tiled acc).

Full inputs in, full output out. Data-parallel over batch N=64 across 8
NeuronCores (8 samples per core); conv weight and centroids replicated.

Structure (per core, 8 samples, 38 pixel-chunks of 128 per sample):
  - x arrives bf16 (host-converted): half the HBM traffic, and every PE
    stream runs at 1 cycle/row (fp32 would be 4).
  - Per chunk: transpose-mode matmul writes x_c^T to a *bf16* PSUM bank
    (8 chunks/bank), evacuated to SBUF by DVE in the 2x packed mode /
    ACT; a second matmul with the same stationary x_c produces logits
    into an f32 PSUM pool large enough for a whole sample (5 banks).
  - slg = logits * inv_s is computed by DVE directly from logits PSUM
    (per-bank ops, step-0 broadcast of inv_s), which both evacuates and
    scales in one 1x pass and releases the logits banks.
  - ss = sum_d x^2: ACT squares, GPSIMD does one pairwise fold (128->64),
    DVE reduces the rest. inv_s = exp(-0.5 ln ss) on ACT.
  - softmax: exp on ACT, Z-reduce + reciprocal on DVE, t = inv_s/Z and
    sbt = et * t broadcasts on GPSIMD.
  - acc matmuls are column-tiled: even chunks accumulate into PSUM
    partitions 0:64, odd chunks into 64:128, running pairwise-concurrent
    in the PE array; the two halves are summed during evacuation.
  - Tail (VLAD normalizations) uses gpsimd.partition_all_reduce for the
    global norm, so no PSUM bank is needed for tiny matmuls.
  PSUM budget: 2 (xt bf16) + 5 (logits f32) + 1 (acc) = 8 banks.
"""

import sys

if "/opt/trn_rl_repo" not in sys.path:
    sys.path.insert(0, "/opt/trn_rl_repo")

import numpy as np
from contextlib import ExitStack

N, D, HW, K = 64, 128, 4800, 64
NCORES = 8
NS = N // NCORES  # samples per core

CHUNKS = [(i * 128, min(128, HW - i * 128)) for i in range((HW + 127) // 128)]
NCH = len(CHUNKS)  # 38: 37 full + one 64-wide
XTS = 130  # xtr row stride (129 used; 130 keeps 4B alignment of chunk rows)

# waves of chunks sharing one xt PSUM bank / one lg PSUM bank
WAVES = [(w * 8, min(8, NCH - w * 8)) for w in range((NCH + 7) // 8)]  # 5 waves
NLG = len(WAVES)

PIPE = 3  # acc matmuls of sample n-PIPE run during round n

_CACHE = {}


def _patch_act_tables():
    """Steer bacc's ACT table-set placement so ln/exp (and the cheap
    square/copy fillers) live in one set: a single ACT_TABLE_LOAD."""
    if _CACHE.get("act_patched"):
        return
    from concourse import bacc, mybir

    orig = bacc.get_activation_tables
    AF = mybir.ActivationFunctionType
    combo = "natural_log_exp_and_others"

    def patched(arch):
        t = {k: set(v) for k, v in orig(arch).items()}
        if combo in t:
            for name in t:
                if name != combo:
                    t[name] = t[name] - {AF.Ln, AF.Exp}
        return t

    bacc.get_activation_tables = patched
    _CACHE["act_patched"] = True


def _build_nc():
    import concourse.tile as tile
    from concourse import bacc, mybir

    _patch_act_tables()

    nc = bacc.Bacc(
        "TRN2",
        target_bir_lowering=False,
        debug=False,
        enable_asserts=False,
        num_devices=NCORES,
    )
    x_ap = nc.dram_tensor(
        "x", [NS, D, HW], mybir.dt.bfloat16, kind="ExternalInput"
    ).ap()
    wt_ap = nc.dram_tensor("wt", [D, K], mybir.dt.bfloat16, kind="ExternalInput").ap()
    cent_ap = nc.dram_tensor(
        "cent", [K, D], mybir.dt.float32, kind="ExternalInput"
    ).ap()
    out_ap = nc.dram_tensor(
        "out", [NS, K, D], mybir.dt.float32, kind="ExternalOutput"
    ).ap()

    with tile.TileContext(nc) as tc:
        with ExitStack() as ctx:
            _body(ctx, tc, out_ap, x_ap, wt_ap, cent_ap)
    nc.compile()
    return nc


def _body(ctx, tc, out_ap, x_ap, wt_ap, cent_ap):
    import concourse.bass as bass
    from concourse import bass_isa, masks, mybir

    nc = tc.nc
    f32 = mybir.dt.float32
    bf16 = mybir.dt.bfloat16
    AF = mybir.ActivationFunctionType
    ALU = mybir.AluOpType
    X_AX = mybir.AxisListType.X

    singles = ctx.enter_context(tc.tile_pool(name="singles", bufs=1))
    xspool = ctx.enter_context(tc.tile_pool(name="xspool", bufs=2))
    xtrpool = ctx.enter_context(tc.tile_pool(name="xtrpool", bufs=PIPE + 1))
    x2pool = ctx.enter_context(tc.tile_pool(name="x2pool", bufs=2))
    slgpool = ctx.enter_context(tc.tile_pool(name="slgpool", bufs=2))
    etpool = ctx.enter_context(tc.tile_pool(name="etpool", bufs=2))
    sbtpool = ctx.enter_context(tc.tile_pool(name="sbtpool", bufs=PIPE))
    smalls = ctx.enter_context(tc.tile_pool(name="smalls", bufs=2))
    tails = ctx.enter_context(tc.tile_pool(name="tails", bufs=1))
    pp_xt = ctx.enter_context(tc.tile_pool(name="pp_xt", bufs=2, space="PSUM"))
    pp_lg = ctx.enter_context(tc.tile_pool(name="pp_lg", bufs=NLG, space="PSUM"))
    pp_acc = ctx.enter_context(tc.tile_pool(name="pp_acc", bufs=1, space="PSUM"))

    def bcast(ap, n):
        # append a step-0 free dim: [..., n] broadcast view
        return bass.AP(tensor=ap.tensor, offset=ap.offset, ap=list(ap.ap) + [[0, n]])

    def mid_bcast(ap, n):
        # [p, f] -> [p, n, f] with step-0 middle dim
        return bass.AP(
            tensor=ap.tensor,
            offset=ap.offset,
            ap=[ap.ap[0], [0, n]] + list(ap.ap[1:]),
        )

    # constants
    identb = singles.tile([128, 128], bf16)
    masks.make_identity(nc, identb[:])
    wtb = singles.tile([D, K], bf16)
    nc.sync.dma_start(out=wtb[:], in_=wt_ap[:])
    cent_s = singles.tile([K, D], f32)
    nc.sync.dma_start(out=cent_s[:], in_=cent_ap[:])

    state = {}  # n -> dict of live tiles
    cstate = {}  # n -> acc psum tile

    def emit_load(n):
        xs = xspool.tile([D, HW], bf16, tag="xs", name="xs")
        nc.sync.dma_start(out=xs[:, 0 : HW // 2], in_=x_ap[n, :, 0 : HW // 2])
        nc.sync.dma_start(out=xs[:, HW // 2 :], in_=x_ap[n, :, HW // 2 :])
        return xs

    # ---- softmax part A of sample m, split into pieces interleaved with
    # the next round's waves (slg / exp / zz / recip / tsc) ----
    def sm_slg(m):
        st = state[m]
        is_ = st["is_"]
        slg = slgpool.tile([128, NCH, K], bf16, tag="slg", name="slg")
        st["slg"] = slg
        # slg = lg * inv_s straight out of PSUM (evac + scale in one pass);
        # releases lg bank t for the new round's logits matmuls
        for t, lg_p in enumerate(st["lgp"]):
            c0, wn = WAVES[t]
            nc.vector.tensor_tensor(
                out=slg[:, c0 : c0 + wn, :],
                in0=lg_p[:, 0:wn, :],
                in1=bcast(is_[:, c0 : c0 + wn], K),
                op=ALU.mult,
            )
        st.pop("lgp")
        st["et"] = etpool.tile([128, NCH, K], bf16, tag="et", name="et")
        st["zz"] = smalls.tile([128, NCH], f32, tag="zz", name="zz")

    SMH = 2 * (NCH // 4)  # 18

    def sm_exp(m, h):
        st = state[m]
        h0, h1 = (0, SMH) if h == 0 else (SMH, NCH)
        nc.scalar.activation(
            st["et"][:, h0:h1, :], st["slg"][:, h0:h1, :], AF.Exp
        )

    def sm_zz(m, h):
        st = state[m]
        h0, h1 = (0, SMH) if h == 0 else (SMH, NCH)
        nc.vector.tensor_reduce(
            out=st["zz"][:, h0:h1], in_=st["et"][:, h0:h1, :], axis=X_AX, op=ALU.add
        )

    def sm_tsc(m):
        st = state[m]
        rr = smalls.tile([128, NCH], f32, tag="rr", name="rr")
        tsc = smalls.tile([128, NCH], bf16, tag="tsc", name="tsc")
        st["tsc"] = tsc
        nc.vector.reciprocal(rr[:], st["zz"][:])
        nc.gpsimd.tensor_tensor(out=tsc[:], in0=st["is_"], in1=rr[:], op=ALU.mult)

    # ---- softmax part B of sample m (runs at round m+2 start: ready work
    # for GPSIMD while everything else in the round is still blocked) ----
    def sm_sbt(m):
        st = state[m]
        et, tsc = st["et"], st["tsc"]
        sbt = sbtpool.tile([128, NCH, K], bf16, tag="sbt", name="sbt")
        st["sbt"] = sbt
        for h0, h1 in ((0, SMH), (SMH, NCH)):
            nc.gpsimd.tensor_tensor(
                out=sbt[:, h0:h1, :],
                in0=et[:, h0:h1, :],
                in1=bcast(tsc[:, h0:h1], K),
                op=ALU.mult,
            )

    # ---- transpose waves of sample n ----
    def emit_wave(n, w):
        st = state[n]
        xs, xtr = st["xs"], st["xtr"]
        c0, wn = WAVES[w]
        xt_p = pp_xt.tile([128, 8, 128], bf16, tag="xt", name="xt")
        for j in range(wn):
            p0, cw = CHUNKS[c0 + j]
            nc.tensor.transpose(xt_p[:cw, j, :], xs[:, p0 : p0 + cw], identb[:])
        dst = xtr[:, c0 : c0 + wn, 0:128]
        src = xt_p[:, 0:wn, :]
        if w % 2 == 0:
            nc.vector.tensor_copy(dst, src)
        else:
            nc.scalar.copy(dst, src)

    def emit_lg_mms(n):
        st = state[n]
        xs = st["xs"]
        lgp = []
        for t in range(NLG):
            c0, wn = WAVES[t]
            lg_p = pp_lg.tile([128, 8, K], f32, tag="lg", name="lg")
            for j in range(wn):
                p0, cw = CHUNKS[c0 + j]
                nc.tensor.matmul(
                    lg_p[:cw, j, :],
                    lhsT=xs[:, p0 : p0 + cw],
                    rhs=wtb[:],
                    start=True,
                    stop=True,
                )
            lgp.append(lg_p)
        st["lgp"] = lgp

    # ---- ss chain of sample n (round tail): is_ ready for round n+1 ----
    SSH = NCH // 2  # 19

    def ss_alloc(n):
        st = state[n]
        st["x2"] = x2pool.tile([128, NCH, 128], bf16, tag="x2", name="x2")
        st["t64"] = x2pool.tile([128, NCH, 64], bf16, tag="t64", name="t64")
        st["ss"] = smalls.tile([128, NCH], f32, tag="ss", name="ss")

    def ss_sq(n, h):
        st = state[n]
        h0, h1 = (0, SSH) if h == 0 else (SSH, NCH)
        nc.scalar.activation(
            st["x2"][:, h0:h1, :], st["xtr"][:, h0:h1, 0:128], AF.Square
        )

    def ss_fold(n, h):
        st = state[n]
        h0, h1 = (0, SSH) if h == 0 else (SSH, NCH)
        nc.gpsimd.tensor_tensor(
            out=st["t64"][:, h0:h1, :],
            in0=st["x2"][:, h0:h1, 0:64],
            in1=st["x2"][:, h0:h1, 64:128],
            op=ALU.add,
        )

    def ss_red(n, h):
        st = state[n]
        h0, h1 = (0, SSH) if h == 0 else (SSH, NCH)
        nc.vector.tensor_reduce(
            out=st["ss"][:, h0:h1], in_=st["t64"][:, h0:h1, :], axis=X_AX, op=ALU.add
        )

    def ss_finish(n):
        st = state[n]
        ss = st["ss"]
        lns = smalls.tile([128, NCH], f32, tag="lns", name="lns")
        is_ = smalls.tile([128, NCH], f32, tag="is", name="is")
        st["is_"] = is_
        # inv_s = exp(-0.5*ln(ss)); Ln+Exp share one ACT table set
        nc.scalar.activation(lns[:], ss[:], AF.Ln)
        nc.scalar.activation(is_[:], lns[:], AF.Exp, scale=-0.5)
        # s-col: xtr[:, c, 128] = ss * inv_s = ||x_p||
        nc.gpsimd.tensor_tensor(
            out=st["xtr"][:, :, 128], in0=ss[:], in1=is_[:], op=ALU.mult
        )

    # ---- col-tiled acc matmuls of sample m ----
    def emit_acc_chunks(m, c0, c1):
        st = state[m]
        xtr, sbt = st["xtr"], st["sbt"]
        if m not in cstate:
            cstate[m] = pp_acc.tile([128, 129], f32, tag="acc", name="acc")
        acc_p = cstate[m]
        for c in range(c0, min(c1, NCH)):
            p0, cw = CHUNKS[c]
            half = c % 2
            nc.tensor.matmul(
                acc_p[64 * half : 64 * half + 64, :],
                lhsT=sbt[:cw, c, :],
                rhs=xtr[:cw, c, 0:129],
                start=(c < 2),
                stop=(c >= NCH - 2),
                tile_position=(0, 64 * half),
            )

    agg_all = tails.tile([K, NS, D], f32)
    ssa_all = tails.tile([K, NS], f32)

    def finish_acc(m):
        acc_p = cstate.pop(m)
        # agg = even-chunk half + odd-chunk half; same for sum_sa column
        nc.scalar.copy(agg_all[:, m, :], acc_p[0:64, 0:D])
        nc.vector.tensor_tensor(
            out=agg_all[:, m, :],
            in0=agg_all[:, m, :],
            in1=acc_p[64:128, 0:D],
            op=ALU.add,
        )
        nc.scalar.copy(ssa_all[:, m : m + 1], acc_p[0:64, 128:129])
        nc.vector.tensor_tensor(
            out=ssa_all[:, m : m + 1],
            in0=ssa_all[:, m : m + 1],
            in1=acc_p[64:128, 128:129],
            op=ALU.add,
        )
        state.pop(m)

    def emit_tail(n0, n1):
        nn = n1 - n0
        agg_h = agg_all[:, n0:n1, :]
        ssa_h = ssa_all[:, n0:n1]
        vl = tails.tile([K, nn, D], f32, tag=f"t_vl{n0}", name="vl")
        vsq = tails.tile([K, nn, D], f32, tag=f"t_vsq{n0}", name="vsq")
        q = tails.tile([K, nn], f32, tag=f"t_q{n0}", name="q")
        qm = tails.tile([K, nn], f32, tag=f"t_qm{n0}", name="qm")
        lq = tails.tile([K, nn], f32, tag=f"t_lq{n0}", name="lq")
        isq = tails.tile([K, nn], f32, tag=f"t_isq{n0}", name="isq")
        isq2 = tails.tile([K, nn], f32, tag=f"t_isq2{n0}", name="isq2")
        u = tails.tile([K, nn], f32, tag=f"t_u{n0}", name="u")
        g = tails.tile([K, nn], f32, tag=f"t_g{n0}", name="g")
        gm = tails.tile([K, nn], f32, tag=f"t_gm{n0}", name="gm")
        lgm = tails.tile([K, nn], f32, tag=f"t_lgm{n0}", name="lgm")
        gis = tails.tile([K, nn], f32, tag=f"t_gis{n0}", name="gis")
        sall = tails.tile([K, nn], f32, tag=f"t_s{n0}", name="sall")
        vf = tails.tile([K, nn, D], f32, tag=f"t_vf{n0}", name="vf")

        # vl = agg - ssa * cent
        nc.gpsimd.tensor_tensor(
            out=vl[:], in0=bcast(ssa_h, D), in1=mid_bcast(cent_s[:], nn), op=ALU.mult
        )
        nc.vector.tensor_tensor(out=vl[:], in0=agg_h, in1=vl[:], op=ALU.subtract)
        # q = rowsum(vl^2) per (k, n)
        nc.scalar.activation(vsq[:], vl[:], AF.Square)
        nc.vector.tensor_reduce(out=q[:], in_=vsq[:], axis=X_AX, op=ALU.add)
        nc.vector.tensor_scalar_max(qm[:], q[:], 1e-24)
        nc.scalar.activation(lq[:], qm[:], AF.Ln)
        nc.scalar.activation(isq[:], lq[:], AF.Exp, scale=-0.5)
        # g[n] = sum_k q_k * isq_k^2, all-reduced across partitions
        nc.vector.tensor_tensor(out=isq2[:], in0=isq[:], in1=isq[:], op=ALU.mult)
        nc.vector.tensor_tensor(out=u[:], in0=q[:], in1=isq2[:], op=ALU.mult)
        nc.gpsimd.partition_all_reduce(
            g[:], u[:], channels=K, reduce_op=bass_isa.ReduceOp.add
        )
        nc.vector.tensor_scalar_max(gm[:], g[:], 1e-24)
        nc.scalar.activation(lgm[:], gm[:], AF.Ln)
        nc.scalar.activation(gis[:], lgm[:], AF.Exp, scale=-0.5)
        # s = isq * gis; vf = vl * s
        nc.vector.tensor_tensor(out=sall[:], in0=isq[:], in1=gis[:], op=ALU.mult)
        nc.gpsimd.tensor_tensor(
            out=vf[:], in0=vl[:], in1=bcast(sall[:], D), op=ALU.mult
        )
        nc.sync.dma_start(
            out=out_ap.rearrange("n k d -> k n d")[:, n0:n1, :], in_=vf[:]
        )

    # ---- main schedule ----
    state[0] = {"xs": emit_load(0)}
    acc_per_wave = (NCH + len(WAVES) - 1) // len(WAVES)

    def round_n(n):
        if n + 1 < NS:
            state[n + 1] = {"xs": emit_load(n + 1)}
        state[n]["xtr"] = xtrpool.tile(
            [128, NCH, XTS], bf16, tag="xtr", name="xtr"
        )
        # ready-at-round-start work first: acc evac of n-PIPE-1 (ACT/DVE),
        # sbt of n-2 (GPSIMD), slg of n-1 (DVE; frees the lg banks this
        # round's logits matmuls need)
        if n >= PIPE + 1:
            finish_acc(n - PIPE - 1)
        if n >= 2:
            sm_sbt(n - 2)
        if n >= 1:
            sm_slg(n - 1)
            sm_exp(n - 1, 0)
        # waves with softmax-of-n-1 pieces woven between the evacs and the
        # acc matmuls of n-PIPE spread across the PE stream
        def accp(w):
            if n >= PIPE:
                emit_acc_chunks(n - PIPE, w * acc_per_wave, (w + 1) * acc_per_wave)

        emit_wave(n, 0)
        accp(0)
        if n >= 1:
            sm_zz(n - 1, 0)
            sm_exp(n - 1, 1)
        emit_wave(n, 1)
        accp(1)
        emit_wave(n, 2)
        accp(2)
        if n >= 1:
            sm_zz(n - 1, 1)
            sm_tsc(n - 1)
        emit_wave(n, 3)
        accp(3)
        emit_wave(n, 4)
        accp(4)
        emit_lg_mms(n)
        # ss chain of n (is_ ready for round n+1's softmax block)
        ss_alloc(n)
        ss_sq(n, 0)
        ss_fold(n, 0)
        ss_sq(n, 1)
        ss_red(n, 0)
        ss_fold(n, 1)
        ss_red(n, 1)
        ss_finish(n)

    for n in range(NS):
        round_n(n)

    # drain: softmax of the last two samples, remaining accs, tail
    finish_acc(NS - PIPE - 1)
    sm_sbt(NS - 2)
    m = NS - 1
    sm_slg(m)
    sm_exp(m, 0)
    sm_zz(m, 0)
    sm_exp(m, 1)
    emit_acc_chunks(NS - PIPE, 0, NCH)
    finish_acc(NS - PIPE)
    sm_zz(m, 1)
    sm_tsc(m)
    sm_sbt(m)
    for mm in range(NS - PIPE + 1, NS):
        emit_acc_chunks(mm, 0, NCH)
        finish_acc(mm)
    emit_tail(0, NS)


def kernel(x, conv_w, centroids):
    import ml_dtypes
    from concourse.bass_utils import run_bass_kernel_spmd

    if "nc" not in _CACHE:
        _CACHE["nc"] = _build_nc()
    nc = _CACHE["nc"]

    x = np.ascontiguousarray(
        np.asarray(x, dtype=np.float32).reshape(N, D, HW).astype(ml_dtypes.bfloat16)
    )
    wt = np.ascontiguousarray(
        np.asarray(conv_w, dtype=np.float32).T.astype(ml_dtypes.bfloat16)
    )
    cent = np.ascontiguousarray(np.asarray(centroids, dtype=np.float32))
    in_maps = [
        {"x": x[i * NS : (i + 1) * NS], "wt": wt, "cent": cent} for i in range(NCORES)
    ]
    res = run_bass_kernel_spmd(nc, in_maps, core_ids=list(range(NCORES))).results
    out = np.concatenate([r["out"].reshape(NS, K * D) for r in res], axis=0)
    return out


if __name__ == "__main__":
    rng = np.random.default_rng(0)
    xs = rng.standard_normal((N, D, 60, 80), dtype=np.float32)
    cw = (rng.standard_normal((K, D)) * 0.1).astype(np.float32)
    ct = rng.random((K, D), dtype=np.float32)
    o = kernel(x=xs, conv_w=cw, centroids=ct)
    print("kernel out", o.shape, o.dtype, np.abs(o).max())
